# revision 1
# baseline (speedup 1.0000x reference)
"""Self-contained FAConv GNN kernel for 8 trn2 NeuronCores (dev version)."""
import os
import numpy as np
from gnn_lib import Cfg, build_plan, build_kernel, prep_inputs
from concourse.bass_utils import run_bass_kernel_spmd


def kernel(**inputs):
    cfg = Cfg()
    meta, per_core = build_plan(inputs["edge_index"], cfg)
    import time
    t0 = time.time()
    nc = build_kernel(cfg, meta)
    print(f"[kernel] build {time.time()-t0:.1f}s, total_sub={meta['total_sub']}")
    in_maps = prep_inputs(inputs, cfg, meta, per_core)
    trace = os.environ.get("KERNEL_TRACE", "0") == "1"
    t0 = time.time()
    res = run_bass_kernel_spmd(nc, in_maps, core_ids=list(range(cfg.NCORES)),
                               trace=trace)
    print(f"[kernel] run (incl neuronxcc compile) {time.time()-t0:.1f}s")
    if res.exec_time_ns is not None:
        print(f"HW exec time: {res.exec_time_ns} ns")
    emb = np.concatenate([res.results[c]["emb"] for c in range(cfg.NCORES)])
    lsm = np.concatenate([res.results[c]["lsm"] for c in range(cfg.NCORES)])
    return emb.astype(np.float32), lsm.astype(np.float32)
